# revision 10
# baseline (speedup 1.0000x reference)
"""Trainium2 Bass kernel for nn_AttentionCrossLayer.

Math: in the reference, softmax over a length-1 axis is exactly 1.0, so
attn == v and q/k/wq/wk are dead code. With x0 the (never-mutated) input,
each layer's gate xw_i = out_i @ cw_i is a fixed linear function of x0:
    xw_i = x0 @ u_i + c_i,   u_i = Wv_i @ (Wo_i @ cw_i),
                             c_i = (bv_i @ Wo_i + bo_i) @ cw_i
and the layer recurrence x += x0 * xw_i + cb_i telescopes to
    out[b, d] = x0[b, d] * (x0[b, :] @ usum + cprime) + cbsum[d]
with usum = sum_i u_i  [D], cprime = 1 + sum_i c_i, cbsum = sum_i cb_i [D].

The tiny weight contractions happen host-side in float64. The device
kernel is one pass over x per core: a fused multiply+row-reduce
(scalar_tensor_tensor with accum_out on the Vector engine) produces the
per-row gate t, then a fused in-place scale-and-add produces the output.

Layout: each SBUF partition holds R=4 *consecutive* DRAM rows, so every
DMA descriptor moves R*4KB contiguous on both sides (descriptor-
throughput is the bottleneck at 4KB/row granularity). A tile covers
R*128 rows; the row-dot runs once per R sub-block with its own accum.

Sharding: data-parallel over batch across 8 cores, weights replicated,
no cross-device comms. Raw Bass (no Tile): explicit semaphores; loads on
the sync engine (HWDGE), stores on gpsimd (SWDGE), compute on Vector.
All tiles are SBUF-resident (no slot reuse), one DMA per semaphore.
"""

import numpy as np

L, B, D, H, K = 3, 32768, 1024, 8, 64
N_CORES = 8
B_LOC = B // N_CORES  # 4096 rows per core
P = 128
R = 4  # consecutive DRAM rows per partition
TILE_ROWS = P * R  # 512
N_TILES = B_LOC // TILE_ROWS  # 8

_cache = {}


def _build_program(cprime: float):
    import concourse.bass as bass
    from concourse import mybir

    F32 = mybir.dt.float32
    MUL = mybir.AluOpType.mult
    ADD = mybir.AluOpType.add

    nc = bass.Bass()
    x = nc.declare_dram_parameter("x", [B_LOC, D], F32, isOutput=False)
    u = nc.declare_dram_parameter("u", [1, D], F32, isOutput=False)
    cb = nc.declare_dram_parameter("cb", [1, D], F32, isOutput=False)
    out = nc.declare_dram_parameter("out", [B_LOC, D], F32, isOutput=True)

    u_bcast = bass.AP(tensor=u.ap().tensor, offset=0, ap=[[0, P], [1, D]])
    cb_bcast = bass.AP(tensor=cb.ap().tensor, offset=0, ap=[[0, P], [1, D]])

    def tile_dram(t, i):
        # rows [i*512, (i+1)*512) with partition p taking rows p*R..p*R+R-1:
        # per partition a contiguous R*D-element run.
        return bass.AP(
            tensor=t.ap().tensor,
            offset=i * TILE_ROWS * D,
            ap=[[R * D, P], [1, R * D]],
        )

    with (
        nc.sbuf_tensor([P, D], F32) as ub,
        nc.sbuf_tensor([P, D], F32) as cbb,
        nc.sbuf_tensor([P, N_TILES, R * D], F32) as xt,
        nc.sbuf_tensor([P, 2, D], F32) as oscr,  # throwaway STT main out
        nc.sbuf_tensor([P, N_TILES, R], F32) as tsc,
        # One DMA outstanding per semaphore: a multi-queue DMA increments
        # its sem in fractions of 16, so cumulative waits over a shared
        # sem would fire early. Per-tile sems avoid that entirely.
        nc.semaphore("us") as us,
        nc.semaphore("cm") as cm,    # pass-1 reduces retired
        nc.semaphore("cma") as cma,  # +cprime fix-ups retired
        nc.semaphore("cm2") as cm2,  # pass-2 writes retired
        nc.Block() as block,
    ):
        lds = [nc.alloc_semaphore(f"ld{i}") for i in range(N_TILES)]
        sts = [nc.alloc_semaphore(f"st{i}") for i in range(N_TILES)]

        @block.scalar
        def _(scalar):
            # broadcasts ride the scalar engine's DMA path so the first x
            # loads aren't queued behind them
            scalar.dma_start(out=ub[:, :], in_=u_bcast).then_inc(us, 16)
            scalar.dma_start(out=cbb[:, :], in_=cb_bcast).then_inc(us, 16)

        @block.sync
        def _(sync):
            for i in range(N_TILES):
                if i >= 2:
                    # cap outstanding loads so early tiles finish fast
                    # instead of all loads round-robining to completion
                    # together (which delays first compute by ~15us)
                    sync.wait_ge(lds[i - 2], 16)
                sync.dma_start(out=xt[:, i, :], in_=tile_dram(x, i)).then_inc(
                    lds[i], 16
                )

        @block.vector
        def _(vector):
            vector.wait_ge(us, 32)
            for i in range(N_TILES):
                vector.wait_ge(lds[i], 16)
                for h in range(R):
                    # oscr = x_h * usum ; tsc[h] = sum_free(oscr)
                    nc.vector.scalar_tensor_tensor(
                        out=oscr[:, h % 2, :],
                        in0=xt[:, i, h * D : (h + 1) * D],
                        scalar=1.0,
                        in1=ub[:, :],
                        op0=MUL,
                        op1=MUL,
                        accum_out=tsc[:, i, h : h + 1],
                    ).then_inc(cm, 1)
                # accumulator writebacks must retire before tsc is read
                vector.wait_ge(cm, R * (i + 1))
                nc.vector.tensor_scalar_add(
                    out=tsc[:, i, :], in0=tsc[:, i, :], scalar1=cprime
                ).then_inc(cma, 1)
                vector.wait_ge(cma, i + 1)
                for h in range(R):
                    # in place: x_h <- x_h * t_h + cbsum
                    nc.vector.scalar_tensor_tensor(
                        out=xt[:, i, h * D : (h + 1) * D],
                        in0=xt[:, i, h * D : (h + 1) * D],
                        scalar=tsc[:, i, h : h + 1],
                        in1=cbb[:, :],
                        op0=MUL,
                        op1=ADD,
                    ).then_inc(cm2, 1)

        @block.gpsimd
        def _(gpsimd):
            for i in range(N_TILES):
                gpsimd.wait_ge(cm2, R * (i + 1))
                gpsimd.dma_start(out=tile_dram(out, i), in_=xt[:, i, :]).then_inc(
                    sts[i], 16
                )
            for i in range(N_TILES):
                gpsimd.wait_ge(sts[i], 16)

    return nc


def _precompute(wv, bv, wo, bo, cw, cb):
    """Host-side f64 contraction of the small per-layer weights."""
    usum = np.zeros(D, np.float64)
    cprime = 1.0
    for i in range(L):
        Wv = wv[i].reshape(D, H * K).astype(np.float64)
        Wo = wo[i].reshape(H * K, D).astype(np.float64)
        cwi = cw[i].reshape(D).astype(np.float64)
        wocw = Wo @ cwi
        usum += Wv @ wocw
        cprime += float(bv[i].reshape(H * K).astype(np.float64) @ wocw)
        cprime += float(bo[i].astype(np.float64) @ cwi)
    cbsum = cb.astype(np.float64).sum(axis=0)
    return usum.astype(np.float32), float(np.float32(cprime)), cbsum.astype(np.float32)


def kernel(x, wq, bq, wk, bk, wv, bv, wo, bo, cw, cb):
    from concourse.bass_utils import run_bass_kernel_spmd

    x = np.ascontiguousarray(np.asarray(x, dtype=np.float32))
    usum, cprime, cbsum = _precompute(
        np.asarray(wv), np.asarray(bv), np.asarray(wo), np.asarray(bo),
        np.asarray(cw), np.asarray(cb),
    )

    if cprime not in _cache:
        _cache[cprime] = _build_program(cprime)
    nc = _cache[cprime]

    u2 = usum.reshape(1, D)
    cb2 = cbsum.reshape(1, D)
    in_maps = [
        {"x": x[c * B_LOC : (c + 1) * B_LOC], "u": u2, "cb": cb2}
        for c in range(N_CORES)
    ]
    res = run_bass_kernel_spmd(nc, in_maps, list(range(N_CORES)))
    return np.concatenate([res.results[c]["out"] for c in range(N_CORES)], axis=0)


# revision 17
# speedup vs baseline: 1.1694x; 1.1694x over previous
"""Trainium2 Bass kernel for nn_AttentionCrossLayer.

Math: in the reference, softmax over a length-1 axis is exactly 1.0, so
attn == v and q/k/wq/wk are dead code. With x0 the (never-mutated) input,
each layer's gate xw_i = out_i @ cw_i is a fixed linear function of x0:
    xw_i = x0 @ u_i + c_i,   u_i = Wv_i @ (Wo_i @ cw_i),
                             c_i = (bv_i @ Wo_i + bo_i) @ cw_i
and the layer recurrence x += x0 * xw_i + cb_i telescopes to
    out[b, d] = x0[b, d] * (x0[b, :] @ usum + cprime) + cbsum[d]
with usum = sum_i u_i  [D], cprime = 1 + sum_i c_i, cbsum = sum_i cb_i [D].

The tiny weight contractions happen host-side in float64. The device
kernel is one pass over x per core, 32 row-tiles of [128, 1024]:
  pass 1 (Vector): fused multiply + row-reduce (scalar_tensor_tensor
    with accum_out) -> per-row gate t. cprime rides in a constant
    column appended to x/u so the reduce emits the finished gate.
  pass 2: in-place x <- x * t + cbsum. When cbsum == 0 (the spec fills
    cb with zeros) this is a pure per-row scale, which the Scalar
    engine's activation op does with a per-partition scale AP — the
    Vector engine then only runs pass 1 and compute never gates the
    DMA stream. A general Vector-engine path handles cbsum != 0.

Loads issue from the sync engine (HWDGE) with a small outstanding cap
so the first tiles land quickly; stores issue from GpSimd (SWDGE). All
32 tiles stay SBUF-resident: no slot reuse, no WAR hazards, and one DMA
outstanding per semaphore (a multi-queue DMA increments its semaphore
in fractions of 16, so cumulative waits over a shared sem fire early).

Sharding: data-parallel over batch across 8 cores, weights replicated,
no cross-device comms.
"""

import numpy as np

L, B, D, H, K = 3, 32768, 1024, 8, 64
N_CORES = 8
B_LOC = B // N_CORES  # 4096 rows per core
P = 128
N_TILES = B_LOC // P  # 32
DP = D + 32  # slot stride 4224B = 128B aligned; col D holds the 1.0 constant
LOAD_CAP = 4  # max outstanding x loads

_cache = {}


def _build_program(cprime: float, zero_cb: bool):
    import concourse.bass as bass
    from concourse import mybir

    F32 = mybir.dt.float32
    MUL = mybir.AluOpType.mult
    ADD = mybir.AluOpType.add

    nc = bass.Bass()
    x = nc.declare_dram_parameter("x", [B_LOC, D], F32, isOutput=False)
    u = nc.declare_dram_parameter("u", [1, D], F32, isOutput=False)
    cb = nc.declare_dram_parameter("cb", [1, D], F32, isOutput=False)
    out = nc.declare_dram_parameter("out", [B_LOC, D], F32, isOutput=True)

    u_bcast = bass.AP(tensor=u.ap().tensor, offset=0, ap=[[0, P], [1, D]])
    cb_bcast = bass.AP(tensor=cb.ap().tensor, offset=0, ap=[[0, P], [1, D]])

    with (
        nc.sbuf_tensor([P, D + 1], F32) as ub,  # [:, :D]=usum, [:, D]=cprime
        nc.sbuf_tensor([P, D], F32) as cbb,
        nc.sbuf_tensor([P, N_TILES, DP], F32) as xt,  # [:, i, D] = 1.0
        nc.sbuf_tensor([P, 2, D + 1], F32) as oscr,  # throwaway STT main out
        nc.sbuf_tensor([P, N_TILES, 1], F32) as tsc,
        nc.semaphore("us") as us,
        nc.semaphore("cm") as cm,    # pass-1 reduces retired (Vector)
        nc.semaphore("cm2") as cm2,  # pass-2 writes retired
        nc.Block() as block,
    ):
        lds = [nc.alloc_semaphore(f"ld{i}") for i in range(N_TILES)]
        sts = [nc.alloc_semaphore(f"st{i}") for i in range(N_TILES)]

        @block.scalar
        def _(scalar):
            # broadcasts ride the scalar engine's DMA path so the first x
            # loads aren't queued behind them
            scalar.dma_start(out=ub[:, 0:D], in_=u_bcast).then_inc(us, 16)
            if not zero_cb:
                scalar.dma_start(out=cbb[:, :], in_=cb_bcast).then_inc(us, 16)
            else:
                # pass 2 on the Scalar engine: x <- x * t (cbsum == 0).
                # waiting on cm also orders us after Vector's memsets
                # (zbias, constant columns), which precede its first op1.
                for i in range(N_TILES):
                    scalar.wait_ge(cm, i + 1)
                    nc.scalar.mul(
                        out=xt[:, i, 0:D],
                        in_=xt[:, i, 0:D],
                        mul=tsc[:, i, :],
                    ).then_inc(cm2, 1)

        @block.sync
        def _(sync):
            for i in range(N_TILES):
                if i >= LOAD_CAP:
                    sync.wait_ge(lds[i - LOAD_CAP], 16)
                sync.dma_start(
                    out=xt[:, i, 0:D], in_=x[i * P : (i + 1) * P, :]
                ).then_inc(lds[i], 16)

        @block.vector
        def _(vector):
            # constants: 1.0 column in every tile slot, cprime in ub,
            # zero activation bias
            nc.vector.memset(xt[:, :, D : D + 1], 1.0)
            nc.vector.memset(ub[:, D : D + 1], cprime)
            vector.wait_ge(us, 16 if zero_cb else 32)
            for i in range(N_TILES):
                vector.wait_ge(lds[i], 16)
                # oscr = x' * u' ; t_i = sum_free = x.usum + cprime
                nc.vector.scalar_tensor_tensor(
                    out=oscr[:, i % 2, :],
                    in0=xt[:, i, 0 : D + 1],
                    scalar=1.0,
                    in1=ub[:, :],
                    op0=MUL,
                    op1=MUL,
                    accum_out=tsc[:, i, :],
                ).then_inc(cm, 1)
                if not zero_cb:
                    # accumulator writeback must retire before t is read
                    vector.wait_ge(cm, i + 1)
                    # in place: x <- x * t + cbsum
                    nc.vector.scalar_tensor_tensor(
                        out=xt[:, i, 0:D],
                        in0=xt[:, i, 0:D],
                        scalar=tsc[:, i, :],
                        in1=cbb[:, :],
                        op0=MUL,
                        op1=ADD,
                    ).then_inc(cm2, 1)

        @block.gpsimd
        def _(gpsimd):
            for i in range(N_TILES):
                gpsimd.wait_ge(cm2, i + 1)
                gpsimd.dma_start(
                    out=out[i * P : (i + 1) * P, :], in_=xt[:, i, 0:D]
                ).then_inc(sts[i], 16)
            for i in range(N_TILES):
                gpsimd.wait_ge(sts[i], 16)

    return nc


def _precompute(wv, bv, wo, bo, cw, cb):
    """Host-side f64 contraction of the small per-layer weights."""
    usum = np.zeros(D, np.float64)
    cprime = 1.0
    for i in range(L):
        Wv = wv[i].reshape(D, H * K).astype(np.float64)
        Wo = wo[i].reshape(H * K, D).astype(np.float64)
        cwi = cw[i].reshape(D).astype(np.float64)
        wocw = Wo @ cwi
        usum += Wv @ wocw
        cprime += float(bv[i].reshape(H * K).astype(np.float64) @ wocw)
        cprime += float(bo[i].astype(np.float64) @ cwi)
    cbsum = cb.astype(np.float64).sum(axis=0)
    return usum.astype(np.float32), float(np.float32(cprime)), cbsum.astype(np.float32)


def kernel(x, wq, bq, wk, bk, wv, bv, wo, bo, cw, cb):
    from concourse.bass_utils import run_bass_kernel_spmd

    x = np.ascontiguousarray(np.asarray(x, dtype=np.float32))
    usum, cprime, cbsum = _precompute(
        np.asarray(wv), np.asarray(bv), np.asarray(wo), np.asarray(bo),
        np.asarray(cw), np.asarray(cb),
    )
    zero_cb = not np.any(cbsum)

    key = (cprime, zero_cb)
    if key not in _cache:
        _cache[key] = _build_program(cprime, zero_cb)
    nc = _cache[key]

    u2 = usum.reshape(1, D)
    cb2 = cbsum.reshape(1, D)
    in_maps = [
        {"x": x[c * B_LOC : (c + 1) * B_LOC], "u": u2, "cb": cb2}
        for c in range(N_CORES)
    ]
    res = run_bass_kernel_spmd(nc, in_maps, list(range(N_CORES)))
    return np.concatenate([res.results[c]["out"] for c in range(N_CORES)], axis=0)
